# revision 16
# baseline (speedup 1.0000x reference)
"""BitLinear (ternary-quantized linear) Trainium2 kernel.

Computes: out = x @ dequant(weight).T where dequant is per-group(128)
AbsMean ternary quantization (w_q in {-1,0,+1} times per-group scale).

Strategy (8 NeuronCores, column-parallel / tensor-parallel):
  - weight [O=11008, K=4096] sharded by rows across 8 cores (1376 each).
  - x [T=8192, K] replicated, host-packed to the transposed tile layout
    AND host-cast to fp16, so each t-tile loads as one contiguous 1MB
    DMA straight into the matmul operand layout (no on-chip cast).
  - On-chip dequant per o-tile (128 rows), engine-balanced around the
    measured rates (DVE fp32 ~115G elem/s, ACT ~118G, GpSimd ~73G):
      ACT: sgn=Sign(w), |w| (in-place);  DVE: abs-sum reduce,
      c=(|w|>s/2), cs=c*sgn;
      GpSimd: w_eff=cs*fp16(s);  PE: [o,k]->[k,o] transposes;
      evac of transposes alternates ACT/DVE.
  - Resident weight: 5 ko-major fp16 tiles of 256/256/256/256/352 cols.
  - Adaptive warmup: a simulated production clock (dequant finishes an
    o-tile every ~PROD ns) paces early t-tile visits, each covering the
    column prefix that is ready when it would issue; the not-yet-ready
    suffix of those tiles is covered by catch-up visits interleaved
    into the steady phase.
  - Visits run pool-contiguous matmul runs (ko-major per weight tile),
    evacuate per psum pool into small staging tiles and DMA the pool's
    column span directly (no full-row staging buffer).
  - Per-core output [T, 1376]; host concatenates along O.
"""

import os
from bisect import bisect_right

import numpy as np

import concourse.bass as bass
import concourse.mybir as mybir
import concourse.tile as tile
from concourse import bacc
from concourse.bass_utils import run_bass_kernel_spmd
from concourse.masks import make_identity

P = 128
GROUP = 128
EPS = 1e-8
TB = 8

FULL_B, FULL_S, FULL_K, FULL_O = 4, 2048, 4096, 11008
N_CORES = 8

LAST_RESULT = None  # BassKernelResults of the most recent run (for test.py)

# Weight-tile column widths: one 512-wide tile per psum pool so steady
# rhs streams are 512 cols (half the per-MM issue overhead of 256).
WTILE_COLS = [512, 512, 352]
WTILE_POOL = [0, 1, 2]

# warmup pacing model (ns): o-tile dequant production rate, first-ready
# latency, per-visit fixed overhead, per-column matmul cost (32 ko).
PROD = 17000
LAT0 = 15000
VISIT_OH = 400
COLT = 13.34
DMA_FLOOR = 3300
CATCHUP_EVERY = 2


def build_program(K, T, O_SHARD, mm_dt=mybir.dt.float16):
    assert K % GROUP == 0 and T % P == 0
    KO = K // GROUP
    KH = KO // 2
    n_ttiles = T // P
    o_tiles = [(o0, min(P, O_SHARD - o0)) for o0 in range(0, O_SHARD, P)]
    n_ot = len(o_tiles)
    assert sum(WTILE_COLS) == O_SHARD
    wt_off = [sum(WTILE_COLS[:i]) for i in range(len(WTILE_COLS))]

    nc = bacc.Bacc("TRN2", target_bir_lowering=False, debug=False)
    xt = nc.dram_tensor("xt", [T, K], mm_dt, kind="ExternalInput").ap()
    w = nc.dram_tensor(
        "w", [O_SHARD, K], mybir.dt.float32, kind="ExternalInput"
    ).ap()
    out = nc.dram_tensor(
        "out", [T, O_SHARD], mybir.dt.float32, kind="ExternalOutput"
    ).ap()

    with tile.TileContext(nc) as tc:
        with (
            tc.tile_pool(name="wres", bufs=1) as wres,
            tc.tile_pool(name="const", bufs=1) as constp,
            tc.tile_pool(name="deq32", bufs=3) as deq32,
            tc.tile_pool(name="deq16", bufs=4) as deq16,
            tc.tile_pool(name="tiny", bufs=8) as tiny,
            tc.tile_pool(name="xin", bufs=4) as xin,
            tc.tile_pool(name="stg", bufs=4) as stgp,
            tc.tile_pool(name="ps_tp", bufs=2, space="PSUM") as ps_tp,
            tc.tile_pool(name="ps_a", bufs=2, space="PSUM") as ps_a,
            tc.tile_pool(name="ps_b", bufs=2, space="PSUM") as ps_b,
            tc.tile_pool(name="ps_c", bufs=2, space="PSUM") as ps_c,
        ):
            ps_pools = [ps_a, ps_b, ps_c]
            pool_w = [512, 512, 352]
            pool_lo = [0, 512, 1024]
            # Resident dequantized transposed weight, ko-major per wtile:
            # wbt[wi][p, ko, col] with contiguous columns per ko (fast rhs).
            wbt = [
                wres.tile([P, KO, csz], mm_dt, tag=f"wbt{wi}", name=f"wbt{wi}")
                for wi, csz in enumerate(WTILE_COLS)
            ]
            ident = constp.tile([P, P], mm_dt)
            make_identity(nc, ident)

            # ------------- dequant of one o-tile (in ko-halves) -----------
            def emit_deq(i, nsplit=2):
                o0, osz = o_tiles[i]
                wi = next(
                    j for j, c0 in enumerate(wt_off)
                    if c0 <= o0 < c0 + WTILE_COLS[j]
                )
                lo = o0 - wt_off[wi]
                wsrc = w[o0 : o0 + osz].rearrange("o (ko k) -> o ko k", k=GROUP)
                step = KO // nsplit
                for h in range(nsplit):
                    ka = h * step
                    wt = deq32.tile([P, KH, GROUP], mybir.dt.float32,
                                    tag="wt", name="wt")
                    sgn = deq16.tile([P, KH, GROUP], mm_dt, tag="sgn",
                                     name="sgn")
                    c = deq16.tile([P, KH, GROUP], mm_dt, tag="c", name="c")
                    sums = tiny.tile([P, KH], mybir.dt.float32, tag="sums")
                    tpos = tiny.tile([P, KH], mybir.dt.float32, tag="tpos")
                    s16 = tiny.tile([P, KH], mm_dt, tag="s16")
                    nc.sync.dma_start(wt[:osz, :step], wsrc[:, ka : ka + step])
                    nc.vector.tensor_reduce(
                        sums[:osz, :step], wt[:osz, :step],
                        axis=mybir.AxisListType.X, op=mybir.AluOpType.add,
                        apply_absolute_value=True,
                    )
                    nc.vector.tensor_scalar(
                        tpos[:osz, :step], sums[:osz, :step], 0.5 / GROUP, 0.5 * EPS,
                        mybir.AluOpType.mult, mybir.AluOpType.max,
                    )
                    nc.vector.tensor_scalar(
                        s16[:osz, :step], sums[:osz, :step], 1.0 / GROUP, EPS,
                        mybir.AluOpType.mult, mybir.AluOpType.max,
                    )
                    nc.scalar.activation(
                        sgn[:osz, :step], wt[:osz, :step],
                        mybir.ActivationFunctionType.Sign,
                    )
                    nc.scalar.activation(
                        wt[:osz, :step], wt[:osz, :step],
                        mybir.ActivationFunctionType.Abs,
                    )
                    nc.vector.tensor_tensor(
                        c[:osz, :step], wt[:osz, :step],
                        tpos[:osz, :step, None].to_broadcast((osz, step, GROUP)),
                        mybir.AluOpType.is_gt,
                    )
                    # cs = c*sgn on DVE; GpSimd only does the scale mult
                    nc.vector.tensor_tensor(
                        c[:osz, :step], c[:osz, :step], sgn[:osz, :step],
                        mybir.AluOpType.mult,
                    )
                    nc.gpsimd.tensor_tensor(
                        c[:osz, :step], c[:osz, :step],
                        s16[:osz, :step, None].to_broadcast((osz, step, GROUP)),
                        mybir.AluOpType.mult,
                    )
                    for g in range(step // TB):
                        kb = ka + g * TB
                        ps = ps_tp.tile([P, TB, P], mm_dt, tag="tp")
                        for j in range(TB):
                            nc.tensor.transpose(
                                ps[:, j, :osz], c[:osz, g * TB + j, :],
                                ident[:osz, :osz],
                            )
                        dst = wbt[wi][:, kb : kb + TB, lo : lo + osz]
                        if g == 0:
                            nc.scalar.copy(dst, ps[:, :, :osz])
                        else:
                            nc.vector.tensor_copy(dst, ps[:, :, :osz])

            # ------------- x tile load ------------------------------------
            xt_r = xt.rearrange("(tt p) (ko t) -> tt p ko t", p=P, t=P)

            def load_x(tt):
                xb = xin.tile([P, KO, P], mm_dt, tag="xb")
                nc.sync.dma_start(xb, xt_r[tt])
                return xb

            # ------------- one visit over a column range ------------------
            def emit_visit(tt, col_lo, col_hi, xb=None):
                if xb is None:
                    xb = load_x(tt)
                t0 = tt * P
                for pi in range(3):
                    a = max(col_lo, pool_lo[pi])
                    b = min(col_hi, pool_lo[pi] + pool_w[pi])
                    if a >= b:
                        continue
                    spans = []
                    for wi in range(len(WTILE_COLS)):
                        if WTILE_POOL[wi] != pi:
                            continue
                        wa = max(a, wt_off[wi]) - wt_off[wi]
                        wb_ = min(b, wt_off[wi] + WTILE_COLS[wi]) - wt_off[wi]
                        if wa < wb_:
                            spans.append((wi, wa, wb_))
                    ps = ps_pools[pi].tile(
                        [P, pool_w[pi]], mybir.dt.float32,
                        tag=f"mm{pi}", name=f"mm{pi}",
                    )
                    for si, (wi, wa, wb_) in enumerate(spans):
                        pa = wt_off[wi] - pool_lo[pi]
                        for ko in range(KO):
                            nc.tensor.matmul(
                                ps[:, pa + wa : pa + wb_],
                                lhsT=xb[:, ko, :],
                                rhs=wbt[wi][:, ko, wa:wb_],
                                start=(ko == 0 and si == 0),
                                stop=(ko == KO - 1),
                            )
                    st = stgp.tile([P, 512], mybir.dt.float32, tag="st")
                    nc.scalar.copy(
                        st[:, : b - a], ps[:, a - pool_lo[pi] : b - pool_lo[pi]]
                    )
                    nc.sync.dma_start(
                        out[t0 : t0 + P, a:b], st[:, : b - a]
                    )

            # ------------- emission schedule -------------
            # simulate dequant production to pace the warmup visits
            ready_t = [LAT0 + PROD * i for i in range(n_ot)]
            events = [(ready_t[i] - PROD, 0, ("deq", i)) for i in range(n_ot)]
            clock = float(ready_t[0])
            tt = 0
            warm = []
            while True:
                cols = min(128 * bisect_right(ready_t, clock), O_SHARD)
                if cols >= O_SHARD:
                    break
                if cols == 0:
                    clock = float(ready_t[0])
                    continue
                events.append((clock, 1, ("visit", (tt, cols))))
                warm.append((tt, cols))
                clock += max(cols * COLT + VISIT_OH, DMA_FLOOR)
                tt += 1
            # hoist the first few x loads so the first visits' operands
            # are on-chip by the time the first weight columns are ready
            HOIST = 3
            xpre = {}
            first = True
            for _, _, (kind, arg) in sorted(events, key=lambda e: (e[0], e[1])):
                if kind == "deq":
                    emit_deq(arg, nsplit=4 if arg < 2 else 2)
                    if first:
                        for j in range(HOIST):
                            xpre[j] = load_x(j)
                        first = False
                else:
                    emit_visit(arg[0], 0, arg[1], xb=xpre.pop(arg[0], None))

            # defer the first steady visits' pool-C span: it would stall on
            # the last o-tiles' dequant; cover it via early catch-ups instead
            DEFER_C = 3
            catchups = [(t, 1024) for t in range(tt, tt + DEFER_C)] + list(warm)
            n_steady = n_ttiles - tt
            ci = 0
            for k, t in enumerate(range(tt, n_ttiles)):
                emit_visit(t, 0, 1024 if k < DEFER_C else O_SHARD)
                want = max(0, k + 3 - DEFER_C) * len(catchups) // max(
                    1, n_steady - DEFER_C)
                while ci < min(want, len(catchups)):
                    jt, jc = catchups[ci]
                    emit_visit(jt, jc, O_SHARD)
                    ci += 1
            while ci < len(catchups):
                jt, jc = catchups[ci]
                emit_visit(jt, jc, O_SHARD)
                ci += 1

    nc.compile()
    return nc


def _run(nc, in_maps, trace=False):
    global LAST_RESULT
    res = run_bass_kernel_spmd(
        nc, in_maps, core_ids=list(range(len(in_maps))), trace=trace
    )
    LAST_RESULT = res
    return res


def pack_x(x2d):
    """[T, K] -> packed fp16: H[tt*P+p, ko*G+t] = x2d[tt*P+t, ko*G+p]."""
    T, K = x2d.shape
    x4 = x2d.reshape(T // P, P, K // GROUP, GROUP)  # [tt, t, ko, p]
    return np.ascontiguousarray(
        x4.transpose(0, 3, 2, 1).reshape(T, K).astype(np.float16)
    )


def kernel(x, weight):
    T = FULL_B * FULL_S
    K = FULL_K
    OS = FULL_O // N_CORES  # 1376
    x2d = pack_x(np.asarray(x, dtype=np.float32).reshape(T, K))
    w = np.asarray(weight, dtype=np.float32)

    nc = build_program(K, T, OS)
    in_maps = [
        {"xt": x2d, "w": np.ascontiguousarray(w[c * OS : (c + 1) * OS])}
        for c in range(N_CORES)
    ]
    trace = bool(os.environ.get("BASS_TRACE"))
    res = _run(nc, in_maps, trace=trace)
    full = np.concatenate(
        [res.results[c]["out"] for c in range(N_CORES)], axis=1
    )
    return np.ascontiguousarray(full.reshape(FULL_B, FULL_S, FULL_O))


# revision 18
# speedup vs baseline: 1.0883x; 1.0883x over previous
"""BitLinear (ternary-quantized linear) Trainium2 kernel.

Computes: out = x @ dequant(weight).T where dequant is per-group(128)
AbsMean ternary quantization (w_q in {-1,0,+1} times per-group scale).

Strategy (8 NeuronCores, column-parallel / tensor-parallel):
  - weight [O=11008, K=4096] sharded by rows across 8 cores (1376 each).
  - x [T=8192, K] replicated, host-packed to the transposed tile layout
    AND host-cast to fp16, so each t-tile loads as one contiguous 1MB
    DMA straight into the matmul operand layout (no on-chip cast).
  - On-chip dequant per o-tile (128 rows), engine-balanced around the
    measured rates (DVE fp32 ~115G elem/s, ACT ~118G, GpSimd ~73G):
      ACT: sgn=Sign(w), |w| (in-place);  DVE: abs-sum reduce,
      c=(|w|>s/2), cs=c*sgn;
      GpSimd: w_eff=cs*fp16(s);  PE: [o,k]->[k,o] transposes;
      evac of transposes alternates ACT/DVE.
  - Resident weight: 5 ko-major fp16 tiles of 256/256/256/256/352 cols.
  - Adaptive warmup: a simulated production clock (dequant finishes an
    o-tile every ~PROD ns) paces early t-tile visits, each covering the
    column prefix that is ready when it would issue; the not-yet-ready
    suffix of those tiles is covered by catch-up visits interleaved
    into the steady phase.
  - Visits run pool-contiguous matmul runs (ko-major per weight tile),
    evacuate per psum pool into small staging tiles and DMA the pool's
    column span directly (no full-row staging buffer).
  - Per-core output [T, 1376]; host concatenates along O.
"""

import os
from bisect import bisect_right

import numpy as np

import concourse.bass as bass
import concourse.mybir as mybir
import concourse.tile as tile
from concourse import bacc
from concourse.bass_utils import run_bass_kernel_spmd
from concourse.masks import make_identity

P = 128
GROUP = 128
EPS = 1e-8
TB = 8

FULL_B, FULL_S, FULL_K, FULL_O = 4, 2048, 4096, 11008
N_CORES = 8

LAST_RESULT = None  # BassKernelResults of the most recent run (for test.py)

# Weight-tile column widths: one 512-wide tile per psum pool so steady
# rhs streams are 512 cols (half the per-MM issue overhead of 256).
WTILE_COLS = [512, 512, 352]
WTILE_POOL = [0, 1, 2]

# warmup pacing model (ns): o-tile dequant production rate, first-ready
# latency, per-visit fixed overhead, per-column matmul cost (32 ko).
PROD = 17000
LAT0 = 15000
VISIT_OH = 400
COLT = 13.34
DMA_FLOOR = 3300
CATCHUP_EVERY = 2


def build_program(K, T, O_SHARD, mm_dt=mybir.dt.float16):
    assert K % GROUP == 0 and T % P == 0
    KO = K // GROUP
    KH = KO // 2
    n_ttiles = T // P
    o_tiles = [(o0, min(P, O_SHARD - o0)) for o0 in range(0, O_SHARD, P)]
    n_ot = len(o_tiles)
    assert sum(WTILE_COLS) == O_SHARD
    wt_off = [sum(WTILE_COLS[:i]) for i in range(len(WTILE_COLS))]

    nc = bacc.Bacc("TRN2", target_bir_lowering=False, debug=False)
    xt = nc.dram_tensor("xt", [T, K], mm_dt, kind="ExternalInput").ap()
    w = nc.dram_tensor(
        "w", [O_SHARD, K], mybir.dt.float32, kind="ExternalInput"
    ).ap()
    out = nc.dram_tensor(
        "out", [T, O_SHARD], mybir.dt.float32, kind="ExternalOutput"
    ).ap()

    with tile.TileContext(nc) as tc:
        with (
            tc.tile_pool(name="wres", bufs=1) as wres,
            tc.tile_pool(name="const", bufs=1) as constp,
            tc.tile_pool(name="deq32", bufs=3) as deq32,
            tc.tile_pool(name="deq16", bufs=4) as deq16,
            tc.tile_pool(name="tiny", bufs=8) as tiny,
            tc.tile_pool(name="xin", bufs=4) as xin,
            tc.tile_pool(name="stg", bufs=4) as stgp,
            tc.tile_pool(name="ps_tp", bufs=2, space="PSUM") as ps_tp,
            tc.tile_pool(name="ps_a", bufs=2, space="PSUM") as ps_a,
            tc.tile_pool(name="ps_b", bufs=2, space="PSUM") as ps_b,
            tc.tile_pool(name="ps_c", bufs=2, space="PSUM") as ps_c,
        ):
            ps_pools = [ps_a, ps_b, ps_c]
            pool_w = [512, 512, 352]
            pool_lo = [0, 512, 1024]
            # Resident dequantized transposed weight, ko-major per wtile:
            # wbt[wi][p, ko, col] with contiguous columns per ko (fast rhs).
            wbt = [
                wres.tile([P, KO, csz], mm_dt, tag=f"wbt{wi}", name=f"wbt{wi}")
                for wi, csz in enumerate(WTILE_COLS)
            ]
            ident = constp.tile([P, P], mm_dt)
            make_identity(nc, ident)

            # ------------- dequant of one o-tile (in ko-halves) -----------
            def emit_deq(i, nsplit=2):
                o0, osz = o_tiles[i]
                wi = next(
                    j for j, c0 in enumerate(wt_off)
                    if c0 <= o0 < c0 + WTILE_COLS[j]
                )
                lo = o0 - wt_off[wi]
                wsrc = w[o0 : o0 + osz].rearrange("o (ko k) -> o ko k", k=GROUP)
                step = KO // nsplit
                for h in range(nsplit):
                    ka = h * step
                    wt = deq32.tile([P, KH, GROUP], mybir.dt.float32,
                                    tag="wt", name="wt")
                    sgn = deq16.tile([P, KH, GROUP], mm_dt, tag="sgn",
                                     name="sgn")
                    c = deq16.tile([P, KH, GROUP], mm_dt, tag="c", name="c")
                    sums = tiny.tile([P, KH], mybir.dt.float32, tag="sums")
                    tpos = tiny.tile([P, KH], mybir.dt.float32, tag="tpos")
                    s16 = tiny.tile([P, KH], mm_dt, tag="s16")
                    nc.sync.dma_start(wt[:osz, :step], wsrc[:, ka : ka + step])
                    nc.vector.tensor_reduce(
                        sums[:osz, :step], wt[:osz, :step],
                        axis=mybir.AxisListType.X, op=mybir.AluOpType.add,
                        apply_absolute_value=True,
                    )
                    nc.vector.tensor_scalar(
                        tpos[:osz, :step], sums[:osz, :step], 0.5 / GROUP, 0.5 * EPS,
                        mybir.AluOpType.mult, mybir.AluOpType.max,
                    )
                    nc.vector.tensor_scalar(
                        s16[:osz, :step], sums[:osz, :step], 1.0 / GROUP, EPS,
                        mybir.AluOpType.mult, mybir.AluOpType.max,
                    )
                    nc.scalar.activation(
                        sgn[:osz, :step], wt[:osz, :step],
                        mybir.ActivationFunctionType.Sign,
                    )
                    nc.scalar.activation(
                        wt[:osz, :step], wt[:osz, :step],
                        mybir.ActivationFunctionType.Abs,
                    )
                    nc.vector.tensor_tensor(
                        c[:osz, :step], wt[:osz, :step],
                        tpos[:osz, :step, None].to_broadcast((osz, step, GROUP)),
                        mybir.AluOpType.is_gt,
                    )
                    # cs = c*sgn on DVE; GpSimd only does the scale mult
                    nc.vector.tensor_tensor(
                        c[:osz, :step], c[:osz, :step], sgn[:osz, :step],
                        mybir.AluOpType.mult,
                    )
                    nc.gpsimd.tensor_tensor(
                        c[:osz, :step], c[:osz, :step],
                        s16[:osz, :step, None].to_broadcast((osz, step, GROUP)),
                        mybir.AluOpType.mult,
                    )
                    for g in range(step // TB):
                        kb = ka + g * TB
                        ps = ps_tp.tile([P, TB, P], mm_dt, tag="tp")
                        for j in range(TB):
                            nc.tensor.transpose(
                                ps[:, j, :osz], c[:osz, g * TB + j, :],
                                ident[:osz, :osz],
                            )
                        dst = wbt[wi][:, kb : kb + TB, lo : lo + osz]
                        if g == 0:
                            nc.scalar.copy(dst, ps[:, :, :osz])
                        else:
                            nc.vector.tensor_copy(dst, ps[:, :, :osz])

            # ------------- x tile load ------------------------------------
            xt_r = xt.rearrange("(tt p) (ko t) -> tt p ko t", p=P, t=P)

            def load_x(tt):
                xb = xin.tile([P, KO, P], mm_dt, tag="xb")
                nc.sync.dma_start(xb, xt_r[tt])
                return xb

            # ------------- one visit over a column range ------------------
            def emit_visit(tt, col_lo, col_hi, xb=None):
                if xb is None:
                    xb = load_x(tt)
                t0 = tt * P
                for pi in range(3):
                    a = max(col_lo, pool_lo[pi])
                    b = min(col_hi, pool_lo[pi] + pool_w[pi])
                    if a >= b:
                        continue
                    spans = []
                    for wi in range(len(WTILE_COLS)):
                        if WTILE_POOL[wi] != pi:
                            continue
                        wa = max(a, wt_off[wi]) - wt_off[wi]
                        wb_ = min(b, wt_off[wi] + WTILE_COLS[wi]) - wt_off[wi]
                        if wa < wb_:
                            spans.append((wi, wa, wb_))
                    ps = ps_pools[pi].tile(
                        [P, pool_w[pi]], mybir.dt.float32,
                        tag=f"mm{pi}", name=f"mm{pi}",
                    )
                    for si, (wi, wa, wb_) in enumerate(spans):
                        pa = wt_off[wi] - pool_lo[pi]
                        for ko in range(KO):
                            nc.tensor.matmul(
                                ps[:, pa + wa : pa + wb_],
                                lhsT=xb[:, ko, :],
                                rhs=wbt[wi][:, ko, wa:wb_],
                                start=(ko == 0 and si == 0),
                                stop=(ko == KO - 1),
                            )
                    st = stgp.tile([P, 512], mybir.dt.float32, tag="st")
                    nc.scalar.copy(
                        st[:, : b - a], ps[:, a - pool_lo[pi] : b - pool_lo[pi]]
                    )
                    nc.sync.dma_start(
                        out[t0 : t0 + P, a:b], st[:, : b - a]
                    )

            # ------------- emission schedule -------------
            # simulate dequant production to pace the warmup visits
            ready_t = [LAT0 + PROD * i for i in range(n_ot)]
            events = [(ready_t[i] - PROD, 0, ("deq", i)) for i in range(n_ot)]
            clock = float(ready_t[0])
            tt = 0
            warm = []
            while True:
                cols = min(128 * bisect_right(ready_t, clock), O_SHARD)
                if cols >= O_SHARD:
                    break
                if cols == 0:
                    clock = float(ready_t[0])
                    continue
                events.append((clock, 1, ("visit", (tt, cols))))
                warm.append((tt, cols))
                clock += max(cols * COLT + VISIT_OH, DMA_FLOOR)
                tt += 1
            # hoist the first visits' x loads right after the first
            # o-tile's dequant DMAs so their operands are on-chip by the
            # time the first weight columns are ready
            HOIST = 3
            xpre = {}
            first = True
            for _, _, (kind, arg) in sorted(events, key=lambda e: (e[0], e[1])):
                if kind == "deq":
                    emit_deq(arg, nsplit=4 if arg < 2 else 2)
                    if first:
                        for j in range(HOIST):
                            xpre[j] = load_x(j)
                        first = False
                else:
                    emit_visit(arg[0], 0, arg[1], xb=xpre.pop(arg[0], None))

            # spread catch-ups evenly across the steady visits
            catchups = list(warm)
            n_steady = n_ttiles - tt
            ci = 0
            for k, t in enumerate(range(tt, n_ttiles)):
                emit_visit(t, 0, O_SHARD)
                want = (k + 1) * len(catchups) // n_steady
                while ci < min(want, len(catchups)):
                    jt, jc = catchups[ci]
                    emit_visit(jt, jc, O_SHARD)
                    ci += 1
            while ci < len(catchups):
                jt, jc = catchups[ci]
                emit_visit(jt, jc, O_SHARD)
                ci += 1

    nc.compile()
    return nc


def _run(nc, in_maps, trace=False):
    global LAST_RESULT
    res = run_bass_kernel_spmd(
        nc, in_maps, core_ids=list(range(len(in_maps))), trace=trace
    )
    LAST_RESULT = res
    return res


def pack_x(x2d):
    """[T, K] -> packed fp16: H[tt*P+p, ko*G+t] = x2d[tt*P+t, ko*G+p]."""
    T, K = x2d.shape
    x4 = x2d.reshape(T // P, P, K // GROUP, GROUP)  # [tt, t, ko, p]
    return np.ascontiguousarray(
        x4.transpose(0, 3, 2, 1).reshape(T, K).astype(np.float16)
    )


def kernel(x, weight):
    T = FULL_B * FULL_S
    K = FULL_K
    OS = FULL_O // N_CORES  # 1376
    x2d = pack_x(np.asarray(x, dtype=np.float32).reshape(T, K))
    w = np.asarray(weight, dtype=np.float32)

    nc = build_program(K, T, OS)
    in_maps = [
        {"xt": x2d, "w": np.ascontiguousarray(w[c * OS : (c + 1) * OS])}
        for c in range(N_CORES)
    ]
    trace = bool(os.environ.get("BASS_TRACE"))
    res = _run(nc, in_maps, trace=trace)
    full = np.concatenate(
        [res.results[c]["out"] for c in range(N_CORES)], axis=1
    )
    return np.ascontiguousarray(full.reshape(FULL_B, FULL_S, FULL_O))
